# revision 3
# baseline (speedup 1.0000x reference)
"""Trainium2 Bass kernel for AttnLSTMDecoder (2-layer LSTM + attention + big vocab
projection + log_softmax), data-parallel over batch across 8 NeuronCores.

Self-contained: hardcodes all shapes. kernel(**inputs) takes the full unsharded
inputs (same keys as reference.setup_inputs()) and returns (log_probs, h_out).

Design notes (per-core, SPMD identical program):
 - Full-batch (B=32) LSTM runs redundantly on every core (the recurrence is
   weight-stream bound on the PE, so sharding batch would not make it faster).
   Each core's program slices batch positions 0:4 for attention/l1/l2; the host
   permutes the batch per core so position 0:4 == that core's shard.
 - Recurrent matmuls: orientation out[batch, gates] with 4x PE column tiling
   (tile_position=(0,32g)): 4 gate-type slices stream concurrently on separate
   XBUSes. Input projections and (optional) biases are accumulated into the
   same PSUM tile per step, so gates = x_t@W_ih.T + h@W_hh.T + b in one group.
 - Gate order is host-permuted to (i, f, o, g) so sigmoid covers PSUM
   partitions 0:96 in one ACT op and tanh covers 96:128.
 - h is produced in bf16 and transposed with the DMA xbar (dma_start_transpose)
   into [128, 4, 32] chunks used as the next step's stationary operand.
 - Vocab projection streams host-pretransposed bf16 W_l2.T from HBM once;
   PSUM tiles are evacuated by ScalarE as exp(logits) (bf16) with accum_out
   collecting the softmax denominator; final pass is Ln(expx * (1/S)) which
   equals logits - logsumexp (logits here are small, |x| < ~3, so the
   max-subtraction in the reference is unnecessary numerically).
"""

import numpy as np
import ml_dtypes

import concourse.bass as bass
import concourse.mybir as mybir
import concourse.tile as tile
from concourse import bacc
from concourse.bass_utils import run_bass_kernel_spmd

BF16 = mybir.dt.bfloat16
F32 = mybir.dt.float32
AF = mybir.ActivationFunctionType

N_CORES = 8
B, T, S, H, E = 32, 64, 64, 512, 256
V = 32000
BLOC = B // N_CORES          # 4 batch elements per core for attention onward
NTOK = BLOC * T              # shard tokens, stok = t*BLOC + j
HC = H // 128                # h-dim chunks
EC = E // 128
GATES = 4 * H
VGROUP = 2048

bf16 = ml_dtypes.bfloat16


def _vgroups():
    out, off = [], 0
    while off < V:
        w = min(VGROUP, V - off)
        out.append((off, w))
        off += w
    return out


def _build(has_b0: bool, has_b1: bool):
    vg = _vgroups()
    nc = bacc.Bacc("TRN2", target_bir_lowering=False, debug=False,
                   num_devices=N_CORES)

    def din(name, shape, dt=BF16):
        return nc.dram_tensor(name, shape, dt, kind="ExternalInput").ap()

    xT = din("xT", [E, B * T])                 # tok = t*B + b (b permuted per core)
    wih0T = din("wih0T", [E, GATES])
    whh0T = din("whh0T", [H, GATES])
    wih1T = din("wih1T", [H, GATES])
    whh1T = din("whh1T", [H, GATES])
    hT0i = din("hT0i", [128, HC, B])           # hidden_init[l].T chunked
    hT1i = din("hT1i", [128, HC, B])
    encT = din("encT", [2 * H, NTOK])          # stok = s*BLOC + j
    encN = din("encN", [S, BLOC * 2 * H])      # normal layout per j
    wattnT = din("wattnT", [2 * H, H])
    wl1T = din("wl1T", [3 * H, H])
    wl2T = din("wl2T", [H, V])
    b0T = din("b0T", [1, GATES]) if has_b0 else None
    b1T = din("b1T", [1, GATES]) if has_b1 else None

    olp = nc.dram_tensor("olp", [2, 128, V], F32, kind="ExternalOutput").ap()
    oho = nc.dram_tensor("oho", [2, B, H], F32, kind="ExternalOutput").ap()

    with tile.TileContext(nc) as tc:
        with tc.tile_pool(name="hmpool", bufs=1) as hmpool:
            hmT_sb = hmpool.tile([128, HC, NTOK], BF16, name="hmT", tag="hmT")

            # ================= LSTM + attention + l1 =================
            with tc.tile_pool(name="wpool", bufs=1) as wpool, \
                 tc.tile_pool(name="opool", bufs=1) as opool, \
                 tc.tile_pool(name="cpool", bufs=1) as cpool:
                xT_sb = wpool.tile([128, EC, B * T], BF16)
                nc.sync.dma_start(out=xT_sb[:], in_=xT.rearrange("(c p) n -> p c n", p=128))
                wih0_sb = wpool.tile([128, EC, GATES], BF16)
                nc.sync.dma_start(out=wih0_sb[:], in_=wih0T.rearrange("(c p) n -> p c n", p=128))
                whh0_sb = wpool.tile([128, HC, GATES], BF16)
                nc.sync.dma_start(out=whh0_sb[:], in_=whh0T.rearrange("(c p) n -> p c n", p=128))
                wih1_sb = wpool.tile([128, HC, GATES], BF16)
                nc.sync.dma_start(out=wih1_sb[:], in_=wih1T.rearrange("(c p) n -> p c n", p=128))
                whh1_sb = wpool.tile([128, HC, GATES], BF16)
                nc.sync.dma_start(out=whh1_sb[:], in_=whh1T.rearrange("(c p) n -> p c n", p=128))
                ones_b = None
                b0_sb = b1_sb = None
                if has_b0 or has_b1:
                    ones_b = wpool.tile([1, B], BF16)
                    nc.vector.memset(ones_b[:], 1.0)
                if has_b0:
                    b0_sb = wpool.tile([1, GATES], BF16)
                    nc.sync.dma_start(out=b0_sb[:], in_=b0T[:])
                if has_b1:
                    b1_sb = wpool.tile([1, GATES], BF16)
                    nc.sync.dma_start(out=b1_sb[:], in_=b1T[:])

                outT_sb = opool.tile([128, HC, B * T], BF16)   # layer-1 h, transposed
                hT0_init = wpool.tile([128, HC, B], BF16)
                nc.sync.dma_start(out=hT0_init[:], in_=hT0i[:])
                hT1_init = wpool.tile([128, HC, B], BF16)
                nc.sync.dma_start(out=hT1_init[:], in_=hT1i[:])

                # cell states at partition rows 32:64 (aligned with sigmoid(f))
                c0_sb = cpool.tile([64, H], F32)
                c1_sb = cpool.tile([64, H], F32)
                nc.vector.memset(c0_sb[32:64, :], 0.0)
                nc.vector.memset(c1_sb[32:64, :], 0.0)
                ho_f32 = cpool.tile([64, H], F32)   # rows 0:32 h0(T-1), 32:64 h1(T-1)

                with tc.tile_pool(name="steps", bufs=3) as sp, \
                     tc.tile_pool(name="gpsum", bufs=4, space="PSUM") as gp_pool:

                    hT_prev = [hT0_init, hT1_init]
                    c_sb = [c0_sb, c1_sb]
                    wih_sb = [wih0_sb, wih1_sb]
                    whh_sb = [whh0_sb, whh1_sb]
                    bias_sb = [b0_sb, b1_sb]

                    for t in range(T):
                        h0T_cur = None
                        for lyr in (0, 1):
                            gp = gp_pool.tile([128, H], F32, name=f"gp_{lyr}_{t}", tag="gp")
                            if lyr == 0:
                                n_a = EC
                                a_lhs = lambda k: xT_sb[:, k, B * t:B * t + B]
                            else:
                                n_a = HC
                                a_lhs = lambda k: h0T_cur[:, k, :]
                            for g in range(4):
                                for k in range(n_a):
                                    nc.tensor.matmul(
                                        out=gp[32 * g:32 * g + 32, :],
                                        lhsT=a_lhs(k),
                                        rhs=wih_sb[lyr][:, k, H * g:H * g + H],
                                        start=(k == 0), stop=False,
                                        tile_position=(0, 32 * g),
                                    )
                                for k in range(HC):
                                    nc.tensor.matmul(
                                        out=gp[32 * g:32 * g + 32, :],
                                        lhsT=hT_prev[lyr][:, k, :],
                                        rhs=whh_sb[lyr][:, k, H * g:H * g + H],
                                        start=False,
                                        stop=(k == HC - 1 and bias_sb[lyr] is None),
                                        tile_position=(0, 32 * g),
                                    )
                                if bias_sb[lyr] is not None:
                                    nc.tensor.matmul(
                                        out=gp[32 * g:32 * g + 32, :],
                                        lhsT=ones_b[:, :],
                                        rhs=bias_sb[lyr][:, H * g:H * g + H],
                                        start=False, stop=True,
                                        tile_position=(0, 32 * g),
                                    )
                            # nonlinearities / cell update
                            sig = sp.tile([96, H], F32, name=f"sig_{lyr}_{t}", tag="sig")
                            nc.scalar.activation(out=sig[:], in_=gp[0:96, :], func=AF.Sigmoid)
                            tg = sp.tile([32, H], F32, name=f"tg_{lyr}_{t}", tag="tg")
                            nc.scalar.activation(out=tg[:], in_=gp[96:128, :], func=AF.Tanh)
                            t2 = sp.tile([32, H], F32, name=f"t2_{lyr}_{t}", tag="t2")
                            nc.vector.tensor_mul(out=t2[:], in0=sig[0:32, :], in1=tg[:])
                            t1 = sp.tile([32, H], F32, name=f"t1_{lyr}_{t}", tag="t1")
                            nc.vector.tensor_mul(out=t1[:], in0=sig[32:64, :],
                                                 in1=c_sb[lyr][32:64, :])
                            nc.vector.tensor_add(out=c_sb[lyr][32:64, :], in0=t1[:], in1=t2[:])
                            th = sp.tile([96, H], F32, name=f"th_{lyr}_{t}", tag="th")
                            nc.scalar.activation(out=th[64:96, :], in_=c_sb[lyr][32:64, :],
                                                 func=AF.Tanh)
                            hbf = sp.tile([32, H], BF16, name=f"hbf_{lyr}_{t}", tag="hbf")
                            nc.vector.tensor_mul(out=hbf[:], in0=sig[64:96, :], in1=th[64:96, :])
                            if t == T - 1:
                                nc.vector.tensor_mul(out=ho_f32[32 * lyr:32 * lyr + 32, :],
                                                     in0=sig[64:96, :], in1=th[64:96, :])
                            if lyr == 0:
                                h0T_cur = sp.tile([128, HC, B], BF16,
                                                  name=f"h0T_{t}", tag="h0T")
                                nc.sync.dma_start_transpose(out=h0T_cur[:], in_=hbf[:])
                                hT_prev[0] = h0T_cur
                            else:
                                nc.sync.dma_start_transpose(
                                    out=outT_sb[:, :, B * t:B * t + B], in_=hbf[:])
                                hT_prev[1] = outT_sb[:, :, B * t:B * t + B]

                nc.sync.dma_start(out=oho.rearrange("l b h -> (l b) h"), in_=ho_f32[:])

                # ---------------- attention + l1 ----------------
                with tc.tile_pool(name="attn", bufs=1) as ap_pool, \
                     tc.tile_pool(name="attnps", bufs=1, space="PSUM") as aps:
                    encT_sb = ap_pool.tile([128, (2 * H) // 128, NTOK], BF16)
                    nc.sync.dma_start(out=encT_sb[:],
                                      in_=encT.rearrange("(c p) n -> p c n", p=128))
                    encN_sb = ap_pool.tile([S, BLOC, 2 * H], BF16)
                    nc.sync.dma_start(out=encN_sb[:],
                                      in_=encN.rearrange("s (j e) -> s j e", j=BLOC))
                    wattnT_sb = ap_pool.tile([128, (2 * H) // 128, H], BF16)
                    nc.sync.dma_start(out=wattnT_sb[:],
                                      in_=wattnT.rearrange("(c p) n -> p c n", p=128))
                    wl1T_sb = ap_pool.tile([128, (3 * H) // 128, H], BF16)
                    nc.sync.dma_start(out=wl1T_sb[:],
                                      in_=wl1T.rearrange("(c p) n -> p c n", p=128))

                    apT_sb = ap_pool.tile([128, HC, NTOK], BF16)
                    for m in range(HC):
                        ps = aps.tile([128, NTOK], F32, name=f"apps_{m}", tag="apps")
                        for k in range(8):
                            nc.tensor.matmul(out=ps[:],
                                             lhsT=wattnT_sb[:, k, 128 * m:128 * m + 128],
                                             rhs=encT_sb[:, k, :],
                                             start=(k == 0), stop=(k == 7))
                        nc.scalar.activation(out=apT_sb[:, m, :], in_=ps[:], func=AF.Copy)

                    ones64 = ap_pool.tile([S, 1], BF16)
                    nc.vector.memset(ones64[:], 1.0)
                    ctxT_sb = ap_pool.tile([128, (2 * H) // 128, NTOK], BF16)
                    outT_v = outT_sb[:].rearrange("p c (t b) -> p c t b", b=B)
                    apT_v = apT_sb[:].rearrange("p c (s j) -> p c s j", j=BLOC)
                    ctxT_v = ctxT_sb[:].rearrange("p c (t j) -> p c t j", j=BLOC)
                    for j in range(BLOC):
                        scps = aps.tile([S, T], F32, name=f"scps_{j}", tag="scps")
                        for k in range(HC):
                            nc.tensor.matmul(out=scps[:], lhsT=apT_v[:, k, :, j],
                                             rhs=outT_v[:, k, :, j],
                                             start=(k == 0), stop=(k == HC - 1))
                        esc = ap_pool.tile([S, T], BF16, name=f"esc_{j}", tag="esc")
                        nc.scalar.activation(out=esc[:], in_=scps[:], func=AF.Exp)
                        sps = aps.tile([1, T], F32, name=f"sps_{j}", tag="sps")
                        nc.tensor.matmul(out=sps[:], lhsT=ones64[:], rhs=esc[:],
                                         start=True, stop=True)
                        rec = ap_pool.tile([1, T], BF16, name=f"rec_{j}", tag="rec")
                        with nc.allow_low_precision(reason="attn 1/S scale, bf16 ok"):
                            nc.vector.reciprocal(out=rec[:], in_=sps[:])
                        recb = ap_pool.tile([S, T], BF16, name=f"recb_{j}", tag="recb")
                        nc.gpsimd.partition_broadcast(out_ap=recb[:], in_ap=rec[:])
                        escn = ap_pool.tile([S, T], BF16, name=f"escn_{j}", tag="escn")
                        nc.vector.tensor_mul(out=escn[:], in0=esc[:], in1=recb[:])
                        for m in range(8):
                            cps = aps.tile([128, T], F32, name=f"cps_{j}_{m}", tag="cps")
                            nc.tensor.matmul(out=cps[:],
                                             lhsT=encN_sb[:, j, 128 * m:128 * m + 128],
                                             rhs=escn[:], start=True, stop=True)
                            nc.scalar.activation(out=ctxT_v[:, m, :, j], in_=cps[:],
                                                 func=AF.Copy)

                    for m in range(HC):
                        ps = aps.tile([128, NTOK], F32, name=f"l1ps_{m}", tag="l1ps")
                        for k in range(12):
                            if k < HC:
                                rhs = outT_v[:, k, :, 0:BLOC]
                            else:
                                rhs = ctxT_sb[:, k - HC, :]
                            nc.tensor.matmul(out=ps[:],
                                             lhsT=wl1T_sb[:, k, 128 * m:128 * m + 128],
                                             rhs=rhs, start=(k == 0), stop=(k == 11))
                        nc.scalar.activation(out=hmT_sb[:, m, :], in_=ps[:], func=AF.Tanh)

            # ================= l2 + log_softmax sweep =================
            with tc.tile_pool(name="l2sb", bufs=1) as l2p, \
                 tc.tile_pool(name="wl2pool", bufs=2) as wl2p, \
                 tc.tile_pool(name="stpool", bufs=2) as stp, \
                 tc.tile_pool(name="l2ps", bufs=2, space="PSUM") as lps:
                expx = [l2p.tile([128, V], BF16, name=f"expx{i}", tag=f"expx{i}")
                        for i in range(2)]
                sums = l2p.tile([128, 2 * len(vg)], F32)
                wl2_v = wl2T.rearrange("(c p) v -> p c v", p=128)
                for gi, (voff, w) in enumerate(vg):
                    wc = wl2p.tile([128, HC, VGROUP], BF16, name=f"wl2c_{gi}", tag="wl2c")
                    nc.sync.dma_start(out=wc[:, :, 0:w], in_=wl2_v[:, :, voff:voff + w])
                    for tau in range(2):
                        ps = lps.tile([128, VGROUP], F32,
                                      name=f"l2ps_{gi}_{tau}", tag="l2ps")
                        for k in range(HC):
                            for nn in range(0, w, 512):
                                nw = min(512, w - nn)
                                nc.tensor.matmul(
                                    out=ps[:, nn:nn + nw],
                                    lhsT=hmT_sb[:, k, 128 * tau:128 * tau + 128],
                                    rhs=wc[:, k, nn:nn + nw],
                                    start=(k == 0), stop=(k == HC - 1))
                        nc.scalar.activation(out=expx[tau][:, voff:voff + w],
                                             in_=ps[:, 0:w], func=AF.Exp,
                                             accum_out=sums[:, 2 * gi + tau:2 * gi + tau + 1])
                sums_v = sums[:].rearrange("p (g u) -> p g u", u=2)
                for tau in range(2):
                    s1 = l2p.tile([128, 1], F32, name=f"s1_{tau}", tag=f"s1_{tau}")
                    nc.vector.tensor_reduce(out=s1[:], in_=sums_v[:, :, tau],
                                            axis=mybir.AxisListType.X,
                                            op=mybir.AluOpType.add)
                    rec = l2p.tile([128, 1], F32, name=f"rs_{tau}", tag=f"rs_{tau}")
                    nc.vector.reciprocal(out=rec[:], in_=s1[:])
                    for voff, w in vg:
                        st = stp.tile([128, VGROUP], F32,
                                      name=f"st_{tau}_{voff}", tag="st")
                        nc.scalar.activation(out=st[:, 0:w],
                                             in_=expx[tau][:, voff:voff + w],
                                             func=AF.Ln, scale=rec[:])
                        nc.sync.dma_start(out=olp[tau, :, voff:voff + w], in_=st[:, 0:w])

    nc.compile()
    return nc


_CACHE = {}


def _get_program(has_b0, has_b1):
    key = (has_b0, has_b1, T, V)
    if key not in _CACHE:
        _CACHE[key] = _build(has_b0, has_b1)
    return _CACHE[key]


def _gperm():
    # pytorch gate rows (i, f, g, o) -> device (i, f, o, g)
    return np.concatenate([np.arange(0, H), np.arange(H, 2 * H),
                           np.arange(3 * H, 4 * H), np.arange(2 * H, 3 * H)])


def make_in_maps(input_ids, encoder_outs, hidden_init, emb,
                 W_ih0, W_hh0, b_ih0, b_hh0, W_ih1, W_hh1, b_ih1, b_hh1,
                 W_attn, W_l1, W_l2):
    input_ids = np.asarray(input_ids)
    encoder_outs = np.asarray(encoder_outs, dtype=np.float32)
    hidden_init = np.asarray(hidden_init, dtype=np.float32)
    emb = np.asarray(emb, dtype=np.float32)
    x = emb[input_ids]                                  # [B, T, E]

    b0 = np.asarray(b_ih0, np.float32) + np.asarray(b_hh0, np.float32)
    b1 = np.asarray(b_ih1, np.float32) + np.asarray(b_hh1, np.float32)
    has_b0 = bool(np.any(b0))
    has_b1 = bool(np.any(b1))
    gperm = _gperm()

    def wprep(w):   # [4H, K] -> bf16 [K, 4H] with gate cols permuted
        return np.ascontiguousarray(np.asarray(w, np.float32)[gperm].T).astype(bf16)

    wih0T = wprep(W_ih0)
    whh0T = wprep(W_hh0)
    wih1T = wprep(W_ih1)
    whh1T = wprep(W_hh1)
    wattnT = np.ascontiguousarray(np.asarray(W_attn, np.float32).T).astype(bf16)
    wl1T = np.ascontiguousarray(np.asarray(W_l1, np.float32).T).astype(bf16)
    wl2T = np.ascontiguousarray(np.asarray(W_l2, np.float32).T).astype(bf16)

    in_maps = []
    for cid in range(N_CORES):
        perm = np.concatenate([np.arange(BLOC * cid, BLOC * cid + BLOC),
                               np.arange(0, BLOC * cid),
                               np.arange(BLOC * cid + BLOC, B)])
        xp = x[perm]
        xT = np.ascontiguousarray(xp.transpose(2, 1, 0).reshape(E, T * B)).astype(bf16)
        hT = hidden_init[:, perm, :]
        hT0i = np.ascontiguousarray(hT[0].T.reshape(HC, 128, B).transpose(1, 0, 2)).astype(bf16)
        hT1i = np.ascontiguousarray(hT[1].T.reshape(HC, 128, B).transpose(1, 0, 2)).astype(bf16)
        enc = encoder_outs[BLOC * cid:BLOC * cid + BLOC]
        encT = np.ascontiguousarray(enc.transpose(2, 1, 0).reshape(2 * H, S * BLOC)).astype(bf16)
        encN = np.ascontiguousarray(enc.transpose(1, 0, 2).reshape(S, BLOC * 2 * H)).astype(bf16)
        m = {
            "xT": xT, "wih0T": wih0T, "whh0T": whh0T, "wih1T": wih1T,
            "whh1T": whh1T, "hT0i": hT0i, "hT1i": hT1i, "encT": encT,
            "encN": encN, "wattnT": wattnT, "wl1T": wl1T, "wl2T": wl2T,
        }
        if has_b0:
            m["b0T"] = b0[gperm].reshape(1, GATES).astype(bf16)
        if has_b1:
            m["b1T"] = b1[gperm].reshape(1, GATES).astype(bf16)
        in_maps.append(m)
    return in_maps, has_b0, has_b1


def assemble(results):
    log_probs = np.empty((B, T, V), np.float32)
    for cid in range(N_CORES):
        olp = results[cid]["olp"]              # [2, 128, V], stok = t*BLOC + j
        shard = olp.reshape(2, T // 2, BLOC, V).transpose(2, 0, 1, 3).reshape(BLOC, T, V)
        log_probs[BLOC * cid:BLOC * cid + BLOC] = shard
    h_out = results[0]["oho"]                  # core 0 has identity batch perm
    return log_probs, h_out


def kernel(input_ids, encoder_outs, hidden_init, targets_len, emb,
           W_ih0, W_hh0, b_ih0, b_hh0, W_ih1, W_hh1, b_ih1, b_hh1,
           W_attn, W_l1, W_l2):
    in_maps, has_b0, has_b1 = make_in_maps(
        input_ids, encoder_outs, hidden_init, emb,
        W_ih0, W_hh0, b_ih0, b_hh0, W_ih1, W_hh1, b_ih1, b_hh1,
        W_attn, W_l1, W_l2)
    nc = _get_program(has_b0, has_b1)
    res = run_bass_kernel_spmd(nc, in_maps, list(range(N_CORES)))
    return assemble(res.results)


# revision 20
# speedup vs baseline: 1.5333x; 1.5333x over previous
"""Trainium2 Bass kernel for AttnLSTMDecoder (2-layer LSTM + attention + big vocab
projection + log_softmax), data-parallel over batch across 8 NeuronCores.

Self-contained: hardcodes all shapes. kernel(**inputs) takes the full unsharded
inputs (same keys as reference.setup_inputs()) and returns (log_probs, h_out).

Design notes (per-core, SPMD identical program):
 - Full-batch (B=32) LSTM runs redundantly on every core (the recurrence is
   weight-stream bound on the PE, so sharding batch would not make it faster).
   Each core's program slices batch positions 0:4 for attention/l1/l2; the host
   permutes the batch per core so position 0:4 == that core's shard.
 - Recurrent matmuls: orientation out[batch, gates] with 4x PE column tiling
   (tile_position=(0,32g)): 4 gate-type slices stream concurrently on separate
   XBUSes. Input projections and (optional) biases are accumulated into the
   same PSUM tile per step, so gates = x_t@W_ih.T + h@W_hh.T + b in one group.
 - Gate order is host-permuted to (i, f, o, g) so sigmoid covers PSUM
   partitions 0:96 in one ACT op and tanh covers 96:128.
 - h is produced in bf16 and transposed with the DMA xbar (dma_start_transpose)
   into [128, 4, 32] chunks used as the next step's stationary operand.
 - Vocab projection streams host-pretransposed bf16 W_l2.T from HBM once;
   PSUM tiles are evacuated by ScalarE as exp(logits) (bf16) with accum_out
   collecting the softmax denominator; final pass is Ln(expx * (1/S)) which
   equals logits - logsumexp (logits here are small, |x| < ~3, so the
   max-subtraction in the reference is unnecessary numerically).
"""

import numpy as np
import ml_dtypes

import concourse.bass as bass
import concourse.mybir as mybir
import concourse.tile as tile
from concourse import bacc
from concourse.bass_utils import run_bass_kernel_spmd
from concourse.masks import make_identity

BF16 = mybir.dt.bfloat16
F32 = mybir.dt.float32
AF = mybir.ActivationFunctionType

N_CORES = 8
B, T, S, H, E = 32, 64, 64, 512, 256
V = 32000
BLOC = B // N_CORES          # 4 batch elements per core for attention onward
NTOK = BLOC * T              # shard tokens, stok = t*BLOC + j
HC = H // 128                # h-dim chunks
EC = E // 128
GATES = 4 * H
VGROUP = 2048

bf16 = ml_dtypes.bfloat16


def _vgroups():
    out, off = [], 0
    while off < V:
        w = min(VGROUP, V - off)
        out.append((off, w))
        off += w
    return out


def _build(has_b0: bool, has_b1: bool):
    vg = _vgroups()
    nc = bacc.Bacc("TRN2", target_bir_lowering=False, debug=False,
                   num_devices=N_CORES)

    def din(name, shape, dt=BF16):
        return nc.dram_tensor(name, shape, dt, kind="ExternalInput").ap()

    xT = din("xT", [E, B * T])                 # tok = t*B + b (b permuted per core)
    wih0T = din("wih0T", [E, GATES])
    whh0T = din("whh0T", [H, GATES])
    wih1T = din("wih1T", [H, GATES])
    whh1T = din("whh1T", [H, GATES])
    hT0i = din("hT0i", [128, HC, B])           # hidden_init[l].T chunked
    hT1i = din("hT1i", [128, HC, B])
    encT = din("encT", [2 * H, NTOK])          # stok = s*BLOC + j
    encN = din("encN", [S, BLOC * 2 * H])      # normal layout per j
    wattnT = din("wattnT", [2 * H, H])
    wl1T = din("wl1T", [3 * H, H])
    wl2T = din("wl2T", [H, V])
    b0T = din("b0T", [1, GATES]) if has_b0 else None
    b1T = din("b1T", [1, GATES]) if has_b1 else None

    olp = nc.dram_tensor("olp", [2, 128, V], F32, kind="ExternalOutput").ap()
    oho = nc.dram_tensor("oho", [2, B, H], F32, kind="ExternalOutput").ap()

    with tile.TileContext(nc) as tc:
        with tc.tile_pool(name="hmpool", bufs=1) as hmpool:
            hmT_sb = hmpool.tile([128, HC, NTOK], BF16, name="hmT", tag="hmT")

            # ================= LSTM + attention + l1 =================
            with tc.tile_pool(name="wpool", bufs=1) as wpool, \
                 tc.tile_pool(name="opool", bufs=1) as opool, \
                 tc.tile_pool(name="cpool", bufs=1) as cpool:
                xT_sb = wpool.tile([128, EC, B * T], BF16)
                nc.sync.dma_start(out=xT_sb[:], in_=xT.rearrange("(c p) n -> p c n", p=128))

                # per-gate-group weight tensors: separate SBUF tensors per PE
                # column group so the 4 moving streams can ride separate XBUSes
                def load_gate_groups(dram, nk, name):
                    tiles = []
                    for g in range(4):
                        tl = wpool.tile([128, nk, H], BF16, name=f"{name}_{g}",
                                        tag=f"{name}_{g}")
                        nc.sync.dma_start(
                            out=tl[:],
                            in_=dram.rearrange("(c p) n -> p c n", p=128)[:, :, H * g:H * g + H])
                        tiles.append(tl)
                    return tiles

                wih0_sb = load_gate_groups(wih0T, EC, "wih0")
                whh0_sb = load_gate_groups(whh0T, HC, "whh0")
                wih1_sb = load_gate_groups(wih1T, HC, "wih1")
                whh1_sb = load_gate_groups(whh1T, HC, "whh1")
                ones_b = None
                b0_sb = b1_sb = None
                if has_b0 or has_b1:
                    ones_b = wpool.tile([1, B], BF16)
                    nc.vector.memset(ones_b[:], 1.0)
                if has_b0:
                    b0_sb = wpool.tile([1, GATES], BF16)
                    nc.sync.dma_start(out=b0_sb[:], in_=b0T[:])
                if has_b1:
                    b1_sb = wpool.tile([1, GATES], BF16)
                    nc.sync.dma_start(out=b1_sb[:], in_=b1T[:])

                outT_sb = opool.tile([128, HC, B * T], BF16)   # layer-1 h, transposed
                id32 = wpool.tile([B, B], BF16)
                make_identity(nc, id32[:])
                # attention/l1 operands loaded up-front so the attention phase
                # can start the moment the last LSTM step completes
                encT_sb = wpool.tile([128, (2 * H) // 128, NTOK], BF16)
                nc.sync.dma_start(out=encT_sb[:],
                                  in_=encT.rearrange("(c p) n -> p c n", p=128))
                encN_sb = wpool.tile([S, BLOC, 2 * H], BF16)
                nc.sync.dma_start(out=encN_sb[:],
                                  in_=encN.rearrange("s (j e) -> s j e", j=BLOC))
                wattnT_sb = wpool.tile([128, (2 * H) // 128, H], BF16)
                nc.sync.dma_start(out=wattnT_sb[:],
                                  in_=wattnT.rearrange("(c p) n -> p c n", p=128))
                wl1T_sb = wpool.tile([128, (3 * H) // 128, H], BF16)
                nc.sync.dma_start(out=wl1T_sb[:],
                                  in_=wl1T.rearrange("(c p) n -> p c n", p=128))
                hT0_init = wpool.tile([128, HC, B], BF16)
                nc.sync.dma_start(out=hT0_init[:], in_=hT0i[:])
                hT1_init = wpool.tile([128, HC, B], BF16)
                nc.sync.dma_start(out=hT1_init[:], in_=hT1i[:])

                # cell states at partition rows 32:64 (aligned with sigmoid(f))
                c0_sb = cpool.tile([64, H], BF16)
                c1_sb = cpool.tile([64, H], BF16)
                nc.vector.memset(c0_sb[32:64, :], 0.0)
                nc.vector.memset(c1_sb[32:64, :], 0.0)
                ho_f32 = cpool.tile([64, H], F32)   # rows 0:32 h0(T-1), 32:64 h1(T-1)

                with tc.tile_pool(name="steps", bufs=3) as sp, \
                     tc.tile_pool(name="gpsum", bufs=4, space="PSUM") as gp_pool:

                    hT_prev = [hT0_init, hT1_init]
                    c_sb = [c0_sb, c1_sb]
                    wih_sb = [wih0_sb, wih1_sb]
                    whh_sb = [whh0_sb, whh1_sb]
                    bias_sb = [b0_sb, b1_sb]
                    h0T_tiles = {}

                    def gate_mms(lyr, t):
                        gp = gp_pool.tile([128, H], F32, name=f"gp_{lyr}_{t}", tag="gp")
                        if lyr == 0:
                            n_a = EC
                            a_lhs = lambda k: xT_sb[:, k, B * t:B * t + B]
                        else:
                            n_a = HC
                            a_lhs = lambda k: h0T_tiles[t][:, k, :]
                        for g in range(4):
                            for k in range(n_a):
                                nc.tensor.matmul(
                                    out=gp[32 * g:32 * g + 32, :],
                                    lhsT=a_lhs(k),
                                    rhs=wih_sb[lyr][g][:, k, :],
                                    start=(k == 0), stop=False,
                                    tile_position=(0, 32 * g),
                                )
                            for k in range(HC):
                                nc.tensor.matmul(
                                    out=gp[32 * g:32 * g + 32, :],
                                    lhsT=hT_prev[lyr][:, k, :],
                                    rhs=whh_sb[lyr][g][:, k, :],
                                    start=False,
                                    stop=(k == HC - 1 and bias_sb[lyr] is None),
                                    tile_position=(0, 32 * g),
                                )
                            if bias_sb[lyr] is not None:
                                nc.tensor.matmul(
                                    out=gp[32 * g:32 * g + 32, :],
                                    lhsT=ones_b[:, :],
                                    rhs=bias_sb[lyr][:, H * g:H * g + H],
                                    start=False, stop=True,
                                    tile_position=(0, 32 * g),
                                )
                        return gp

                    def chain(lyr, t, gp):
                        # trp allocated up-front: warm-keeper transposes write its
                        # first column block mid-chain (overwritten by the real
                        # transpose below) purely to keep the PE HAM un-throttled.
                        trp = gp_pool.tile([128, HC * B], BF16,
                                           name=f"tr_{lyr}_{t}", tag="tr")
                        sig = sp.tile([96, H], BF16, name=f"sig_{lyr}_{t}", tag="sig")
                        nc.scalar.activation(out=sig[:], in_=gp[0:96, :], func=AF.Sigmoid)
                        tg = sp.tile([32, H], BF16, name=f"tg_{lyr}_{t}", tag="tg")
                        nc.scalar.activation(out=tg[:], in_=gp[96:128, :], func=AF.Tanh)
                        t1 = sp.tile([32, H], BF16, name=f"t1_{lyr}_{t}", tag="t1")
                        nc.vector.tensor_mul(out=t1[:], in0=sig[32:64, :],
                                             in1=c_sb[lyr][32:64, :])
                        t2 = sp.tile([32, H], BF16, name=f"t2_{lyr}_{t}", tag="t2")
                        nc.vector.tensor_mul(out=t2[:], in0=sig[0:32, :], in1=tg[:])
                        nc.vector.tensor_add(out=c_sb[lyr][32:64, :], in0=t1[:], in1=t2[:])
                        nc.tensor.transpose(out=trp[:, 0:B], in_=t2[:, 0:128],
                                            identity=id32[:])
                        th = sp.tile([96, H], BF16, name=f"th_{lyr}_{t}", tag="th")
                        nc.scalar.activation(out=th[64:96, :], in_=c_sb[lyr][32:64, :],
                                             func=AF.Tanh)
                        hbf = sp.tile([32, H], BF16, name=f"hbf_{lyr}_{t}", tag="hbf")
                        nc.vector.tensor_mul(out=hbf[:], in0=sig[64:96, :], in1=th[64:96, :])
                        if t == T - 1:
                            nc.vector.tensor_mul(out=ho_f32[32 * lyr:32 * lyr + 32, :],
                                                 in0=sig[64:96, :], in1=th[64:96, :])
                        return hbf, trp

                    def transpose_h(lyr, t, hbf, trp):
                        for k in range(HC):
                            nc.tensor.transpose(out=trp[:, B * k:B * k + B],
                                                in_=hbf[:, 128 * k:128 * k + 128],
                                                identity=id32[:])
                        if lyr == 0:
                            h0T = sp.tile([128, HC, B], BF16,
                                          name=f"h0T_{t}", tag="h0T")
                            nc.scalar.activation(
                                out=h0T[:].rearrange("p c b -> p (c b)"),
                                in_=trp[:], func=AF.Copy)
                            h0T_tiles[t] = h0T
                            h0T_tiles.pop(t - 2, None)
                            hT_prev[0] = h0T
                        else:
                            nc.scalar.activation(
                                out=outT_sb[:, :, B * t:B * t + B],
                                in_=trp[:].rearrange("p (c b) -> p c b", b=B),
                                func=AF.Copy)
                            hT_prev[1] = outT_sb[:, :, B * t:B * t + B]

                    # software pipeline: layer 0 runs TWO steps ahead of layer 1,
                    # so every layer-1 op's dependencies were produced in earlier
                    # iterations and nothing trailing ever stalls an engine FIFO;
                    # the only in-iteration wait is L0's own recurrence cycle.
                    for t0 in range(2):
                        gp0 = gate_mms(0, t0)
                        hbf0, trp0 = chain(0, t0, gp0)
                        transpose_h(0, t0, hbf0, trp0)
                    for t in range(T):
                        if t + 2 < T:
                            gp0 = gate_mms(0, t + 2)
                        gp1 = gate_mms(1, t)
                        if t + 2 < T:
                            hbf0, trp0 = chain(0, t + 2, gp0)
                            transpose_h(0, t + 2, hbf0, trp0)
                        hbf1, trp1 = chain(1, t, gp1)
                        transpose_h(1, t, hbf1, trp1)

                nc.sync.dma_start(out=oho.rearrange("l b h -> (l b) h"), in_=ho_f32[:])

                # ---------------- attention + l1 ----------------
                with tc.tile_pool(name="attn", bufs=1) as ap_pool, \
                     tc.tile_pool(name="attnps", bufs=1, space="PSUM") as aps:
                    apT_sb = ap_pool.tile([128, HC, NTOK], BF16)
                    for m in range(HC):
                        ps = aps.tile([128, NTOK], F32, name=f"apps_{m}", tag="apps")
                        for k in range(8):
                            nc.tensor.matmul(out=ps[:],
                                             lhsT=wattnT_sb[:, k, 128 * m:128 * m + 128],
                                             rhs=encT_sb[:, k, :],
                                             start=(k == 0), stop=(k == 7))
                        nc.scalar.activation(out=apT_sb[:, m, :], in_=ps[:], func=AF.Copy)

                    ones64 = ap_pool.tile([S, 1], BF16)
                    nc.vector.memset(ones64[:], 1.0)
                    ctxT_sb = ap_pool.tile([128, (2 * H) // 128, NTOK], BF16)
                    outT_v = outT_sb[:].rearrange("p c (t b) -> p c t b", b=B)
                    apT_v = apT_sb[:].rearrange("p c (s j) -> p c s j", j=BLOC)
                    ctxT_v = ctxT_sb[:].rearrange("p c (t j) -> p c t j", j=BLOC)
                    scps = aps.tile([S, BLOC * T], F32, name="scps", tag="scps")
                    for j in range(BLOC):
                        for k in range(HC):
                            nc.tensor.matmul(out=scps[:, T * j:T * j + T],
                                             lhsT=apT_v[:, k, :, j],
                                             rhs=outT_v[:, k, :, j],
                                             start=(k == 0), stop=(k == HC - 1))
                    esc = ap_pool.tile([S, BLOC * T], BF16, name="esc", tag="esc")
                    nc.scalar.activation(out=esc[:], in_=scps[:], func=AF.Exp)
                    sps = aps.tile([1, BLOC * T], F32, name="sps", tag="sps")
                    nc.tensor.matmul(out=sps[:], lhsT=ones64[:], rhs=esc[:],
                                     start=True, stop=True)
                    rec = ap_pool.tile([1, BLOC * T], BF16, name="rec", tag="rec")
                    with nc.allow_low_precision(reason="attn 1/S scale, bf16 ok"):
                        nc.vector.reciprocal(out=rec[:], in_=sps[:])
                    recb = ap_pool.tile([S, BLOC * T], BF16, name="recb", tag="recb")
                    nc.gpsimd.partition_broadcast(out_ap=recb[:], in_ap=rec[:])
                    escn = ap_pool.tile([S, BLOC * T], BF16, name="escn", tag="escn")
                    nc.vector.tensor_mul(out=escn[:], in0=esc[:], in1=recb[:])
                    for j in range(BLOC):
                        for m in range(8):
                            cps = aps.tile([128, T], F32, name=f"cps_{j}_{m}", tag="cps")
                            nc.tensor.matmul(out=cps[:],
                                             lhsT=encN_sb[:, j, 128 * m:128 * m + 128],
                                             rhs=escn[:, T * j:T * j + T],
                                             start=True, stop=True)
                            nc.scalar.activation(out=ctxT_v[:, m, :, j], in_=cps[:],
                                                 func=AF.Copy)

                    for m in range(HC):
                        ps = aps.tile([128, NTOK], F32, name=f"l1ps_{m}", tag="l1ps")
                        for k in range(12):
                            if k < HC:
                                rhs = outT_v[:, k, :, 0:BLOC]
                            else:
                                rhs = ctxT_sb[:, k - HC, :]
                            nc.tensor.matmul(out=ps[:],
                                             lhsT=wl1T_sb[:, k, 128 * m:128 * m + 128],
                                             rhs=rhs, start=(k == 0), stop=(k == 11))
                        nc.scalar.activation(out=hmT_sb[:, m, :], in_=ps[:], func=AF.Tanh)

            # ================= l2 + log_softmax sweep =================
            with tc.tile_pool(name="l2sb", bufs=1) as l2p, \
                 tc.tile_pool(name="wl2pool", bufs=2) as wl2p, \
                 tc.tile_pool(name="stpool", bufs=2) as stp, \
                 tc.tile_pool(name="l2ps", bufs=2, space="PSUM") as lps:
                expx = [l2p.tile([128, V], BF16, name=f"expx{i}", tag=f"expx{i}")
                        for i in range(2)]
                sums = [l2p.tile([128, len(vg)], F32, name=f"sums{i}", tag=f"sums{i}")
                        for i in range(2)]
                wl2_v = wl2T.rearrange("(c p) v -> p c v", p=128)
                for gi, (voff, w) in enumerate(vg):
                    wc = wl2p.tile([128, HC, VGROUP], BF16,
                                   name=f"wl2c_{gi}", tag="wl2c")
                    nc.sync.dma_start(out=wc[:, :, 0:w],
                                      in_=wl2_v[:, :, voff:voff + w])
                    for tau in range(2):
                        ps = lps.tile([128, VGROUP], F32,
                                      name=f"l2ps_{gi}_{tau}", tag="l2ps")
                        for k in range(HC):
                            for nn in range(0, w, 512):
                                nw = min(512, w - nn)
                                nc.tensor.matmul(
                                    out=ps[:, nn:nn + nw],
                                    lhsT=hmT_sb[:, k, 128 * tau:128 * tau + 128],
                                    rhs=wc[:, k, nn:nn + nw],
                                    start=(k == 0), stop=(k == HC - 1))
                        nc.scalar.activation(out=expx[tau][:, voff:voff + w],
                                             in_=ps[:, 0:w], func=AF.Exp,
                                             accum_out=sums[tau][:, gi:gi + 1])
                for tau in range(2):
                    s1 = l2p.tile([128, 1], F32, name=f"s1_{tau}", tag=f"s1_{tau}")
                    nc.vector.tensor_reduce(out=s1[:], in_=sums[tau][:],
                                            axis=mybir.AxisListType.X,
                                            op=mybir.AluOpType.add)
                    rec = l2p.tile([128, 1], F32, name=f"rs_{tau}", tag=f"rs_{tau}")
                    nc.vector.reciprocal(out=rec[:], in_=s1[:])
                    for voff, w in vg:
                        st = stp.tile([128, VGROUP], F32,
                                      name=f"st_{tau}_{voff}", tag="st")
                        nc.scalar.activation(out=st[:, 0:w],
                                             in_=expx[tau][:, voff:voff + w],
                                             func=AF.Ln, scale=rec[:])
                        nc.sync.dma_start(out=olp[tau, :, voff:voff + w], in_=st[:, 0:w])

    nc.compile()
    return nc


_CACHE = {}


def _get_program(has_b0, has_b1):
    key = (has_b0, has_b1, T, V)
    if key not in _CACHE:
        _CACHE[key] = _build(has_b0, has_b1)
    return _CACHE[key]


def _gperm():
    # pytorch gate rows (i, f, g, o) -> device (i, f, o, g)
    return np.concatenate([np.arange(0, H), np.arange(H, 2 * H),
                           np.arange(3 * H, 4 * H), np.arange(2 * H, 3 * H)])


def make_in_maps(input_ids, encoder_outs, hidden_init, emb,
                 W_ih0, W_hh0, b_ih0, b_hh0, W_ih1, W_hh1, b_ih1, b_hh1,
                 W_attn, W_l1, W_l2):
    input_ids = np.asarray(input_ids)
    encoder_outs = np.asarray(encoder_outs, dtype=np.float32)
    hidden_init = np.asarray(hidden_init, dtype=np.float32)
    emb = np.asarray(emb, dtype=np.float32)
    x = emb[input_ids]                                  # [B, T, E]

    b0 = np.asarray(b_ih0, np.float32) + np.asarray(b_hh0, np.float32)
    b1 = np.asarray(b_ih1, np.float32) + np.asarray(b_hh1, np.float32)
    has_b0 = bool(np.any(b0))
    has_b1 = bool(np.any(b1))
    gperm = _gperm()

    def wprep(w):   # [4H, K] -> bf16 [K, 4H] with gate cols permuted
        return np.ascontiguousarray(np.asarray(w, np.float32)[gperm].T).astype(bf16)

    wih0T = wprep(W_ih0)
    whh0T = wprep(W_hh0)
    wih1T = wprep(W_ih1)
    whh1T = wprep(W_hh1)
    wattnT = np.ascontiguousarray(np.asarray(W_attn, np.float32).T).astype(bf16)
    wl1T = np.ascontiguousarray(np.asarray(W_l1, np.float32).T).astype(bf16)
    wl2T = np.ascontiguousarray(np.asarray(W_l2, np.float32).T).astype(bf16)

    in_maps = []
    for cid in range(N_CORES):
        perm = np.concatenate([np.arange(BLOC * cid, BLOC * cid + BLOC),
                               np.arange(0, BLOC * cid),
                               np.arange(BLOC * cid + BLOC, B)])
        xp = x[perm]
        xT = np.ascontiguousarray(xp.transpose(2, 1, 0).reshape(E, T * B)).astype(bf16)
        hT = hidden_init[:, perm, :]
        hT0i = np.ascontiguousarray(hT[0].T.reshape(HC, 128, B).transpose(1, 0, 2)).astype(bf16)
        hT1i = np.ascontiguousarray(hT[1].T.reshape(HC, 128, B).transpose(1, 0, 2)).astype(bf16)
        enc = encoder_outs[BLOC * cid:BLOC * cid + BLOC]
        encT = np.ascontiguousarray(enc.transpose(2, 1, 0).reshape(2 * H, S * BLOC)).astype(bf16)
        encN = np.ascontiguousarray(enc.transpose(1, 0, 2).reshape(S, BLOC * 2 * H)).astype(bf16)
        m = {
            "xT": xT, "wih0T": wih0T, "whh0T": whh0T, "wih1T": wih1T,
            "whh1T": whh1T, "hT0i": hT0i, "hT1i": hT1i, "encT": encT,
            "encN": encN, "wattnT": wattnT, "wl1T": wl1T, "wl2T": wl2T,
        }
        if has_b0:
            m["b0T"] = b0[gperm].reshape(1, GATES).astype(bf16)
        if has_b1:
            m["b1T"] = b1[gperm].reshape(1, GATES).astype(bf16)
        in_maps.append(m)
    return in_maps, has_b0, has_b1


def assemble(results):
    log_probs = np.empty((B, T, V), np.float32)
    for cid in range(N_CORES):
        olp = results[cid]["olp"]              # [2, 128, V], stok = t*BLOC + j
        shard = olp.reshape(2, T // 2, BLOC, V).transpose(2, 0, 1, 3).reshape(BLOC, T, V)
        log_probs[BLOC * cid:BLOC * cid + BLOC] = shard
    h_out = results[0]["oho"]                  # core 0 has identity batch perm
    return log_probs, h_out


def kernel(input_ids, encoder_outs, hidden_init, targets_len, emb,
           W_ih0, W_hh0, b_ih0, b_hh0, W_ih1, W_hh1, b_ih1, b_hh1,
           W_attn, W_l1, W_l2):
    in_maps, has_b0, has_b1 = make_in_maps(
        input_ids, encoder_outs, hidden_init, emb,
        W_ih0, W_hh0, b_ih0, b_hh0, W_ih1, W_hh1, b_ih1, b_hh1,
        W_attn, W_l1, W_l2)
    nc = _get_program(has_b0, has_b1)
    res = run_bass_kernel_spmd(nc, in_maps, list(range(N_CORES)))
    return assemble(res.results)
